# revision 51
# baseline (speedup 1.0000x reference)
"""Trainium2 Bass kernel for nn_PointTransformerLayer_59674275611307.

Mathematical simplification: in the reference, the attention logits `w` are
broadcast identically across the NSAMPLE axis before the softmax.  Softmax
over identical values is exactly uniform (1/16 each), and the weights sum to
exactly 1, so the grouped weighted sum of values collapses to the values
themselves:

    out = (xv_g * attn).sum(axis=1) == xv == x @ Wv + bv

(verified: rel err ~2e-7 vs the full reference).  Everything else — the q/k
projections, the position MLP, both BN+MLP stacks and the softmax — cancels
out of the output entirely.  The kernel therefore computes the single
(50000,64)@(64,64) matmul + bias, data-parallel over points across 8 cores.

Device strategy (per core, 6400 rows after padding 50000 -> 51200):
  - host packs the core's shard into ONE dram tensor "xtw" [128, 3328]:
    cols 0:64   = Wv stacked twice on the partition dim (for the two PE
                  row-groups), cols 64:128 = bias replicated, cols 128:3328
    = x transposed in chunk-pairs: partition 64*(t&1)+k, column
    128 + (t>>1)*128 + p holds x[p*50 + t, k]  (t = 128-row chunk index,
    p = row-within-chunk).  The contraction dim k becomes the SBUF
    partition dim (the PE contracts over partitions) with zero on-device
    transposes, full-128-partition DMAs, and contiguous descriptors.
  - 50 fp32 matmuls (lhsT = x-chunk.T [64,128] stationary, rhs = Wv [64,64]
    moving), row-packed in even/odd pairs at tile_position (0,0)/(64,0) so
    consecutive chunks run concurrently in disjoint PE row-groups writing
    separate PSUM banks.
  - dummy bf16 matmuls warm the PE HAM clock gate during the input DMAs.
  - DVE adds the bias while copying PSUM -> SBUF; contiguous stores.
"""

import numpy as np

N = 50000
C = 64
NCORES = 8
T = 50                        # 128-row chunks per core
ROWS_PER_CORE = 128 * T       # 6400
N_PAD = NCORES * ROWS_PER_CORE  # 51200
PAIRS = T // 2                # 25
XT_COLS = PAIRS * 128         # 3200
HDR = 128                     # wv (64) + bias (64) header columns
GROUP = 8                     # max chunks per psum group (4 even + 4 odd)


def _group_sizes():
    sizes = [2, 2, 4]
    while sum(sizes) < T:
        sizes.append(min(GROUP, T - sum(sizes)))
    return sizes


GROUP_SIZES = _group_sizes()
GROUP_BOUNDS = [0]
for _s in GROUP_SIZES:
    GROUP_BOUNDS.append(GROUP_BOUNDS[-1] + _s)
# xt load pieces in pairs-of-chunks; piece 0 additionally carries the header
XT_PIECE_PAIRS = [4, 7, 7, 7]
# Output store pieces (columns of the [128, 3200] out-sbuf layout).
OUT_PIECES = [(0, 1024), (1024, 2048), (2048, 3072), (3072, 3200)]
N_WARMUP = 12  # dummy bf16 matmuls to engage the PE HAM clock during DMA-in

TRACE = False          # test.py sets True to collect an NTFF profile
LAST_RESULT = None     # BassKernelResults of the last run (for test.py)

_cache = {}


def _get_compiled():
    if "nc" in _cache:
        return _cache["nc"]

    import concourse.mybir as mybir
    import concourse.tile as tile
    from concourse import bacc
    from concourse.bass import MemorySpace

    f32 = mybir.dt.float32
    bf16 = mybir.dt.bfloat16
    nc = bacc.Bacc("TRN2", target_bir_lowering=False, debug=False,
                   num_devices=NCORES)

    xtw_d = nc.dram_tensor("xtw", [128, HDR + XT_COLS], f32,
                           kind="ExternalInput")
    # Output stored group-contiguous: store s occupies a fully contiguous
    # DRAM region [128, w_s] (HBM writes at 12.8KB partition stride measured
    # only ~190GB/s; contiguous pieces avoid that).  Host reassembles.
    out_d = nc.dram_tensor("out", [128 * XT_COLS], f32,
                           kind="ExternalOutput")

    with tile.TileContext(nc) as tc:
        with (
            tc.tile_pool(name="const", bufs=1) as constp,
            tc.tile_pool(name="xt", bufs=1) as xtp,
            tc.tile_pool(name="outp", bufs=1) as outp,
            tc.tile_pool(name="ps", bufs=3, space=MemorySpace.PSUM) as psp,
        ):
            # PE warmup: dummy bf16 matmuls gated only on a gpsimd memset,
            # so they run during the input-DMA window and the HAM clock
            # gate reaches 8/8 before the real (fp32) matmul stream starts.
            scr = constp.tile([128, 384], bf16, tag="scr")
            nc.gpsimd.memset(scr[:], 0.0)
            ps_w = psp.tile([128, 512], f32, tag="warm", bufs=1)
            for _ in range(N_WARMUP):
                nc.tensor.matmul(ps_w[:, :384], scr[:, :128], scr[:],
                                 start=True, stop=True)

            # Input DMAs: all pieces on the sync HWDGE ring, in consumption
            # order — the ring drains FIFO, so piece 0 (header + first
            # pairs) completes first and the matmul stream starts early.
            # (Splitting across rings makes every piece finish near the end
            # because the SDMA engines round-robin between rings.)
            xt_tiles = []   # (tile, first_pair, col_off_of_first_pair)
            col = 0
            for i, npair in enumerate(XT_PIECE_PAIRS):
                w = npair * 128 + (HDR if i == 0 else 0)
                t_ = xtp.tile([128, w], f32, tag=f"xt{i}", name=f"xt_sb{i}")
                nc.sync.dma_start(t_[:], xtw_d.ap()[:, col:col + w])
                first_pair = 0 if i == 0 else (col - HDR) // 128
                xt_tiles.append((t_, first_pair, HDR if i == 0 else 0))
                col += w
            wv = xt_tiles[0][0][:, 0:C]
            bias = xt_tiles[0][0][:, C:2 * C]

            def lhsT_of(p2, a):
                for t_, first, off in xt_tiles:
                    npair = (t_.shape[1] - off) // 128
                    if first <= p2 < first + npair:
                        local = off + (p2 - first) * 128
                        return t_[64 * a:64 * (a + 1), local:local + 128]
                raise AssertionError(p2)

            out_tiles = []
            for i, (lo, hi) in enumerate(OUT_PIECES):
                out_tiles.append(
                    outp.tile([128, hi - lo], f32, tag=f"out{i}",
                              name=f"out_sb{i}"))

            def out_piece_of(col):
                for i, (lo, hi) in enumerate(OUT_PIECES):
                    if lo <= col < hi:
                        return i, col - lo
                raise AssertionError(col)

            # group sizes ramp up: small first groups let the first store
            # (and the slow ~225GB/s HBM write stream) start ~1.5us earlier
            n_groups = len(GROUP_SIZES)
            for g in range(n_groups):
                t0 = GROUP_BOUNDS[g]
                t1 = GROUP_BOUNDS[g + 1]
                nhalf = (t1 - t0) // 2          # chunks per parity
                ps_e = psp.tile([128, 256], f32, tag="mme")
                ps_o = psp.tile([128, 256], f32, tag="mmo")
                for t in range(t0, t1):
                    a = t & 1
                    p2 = t >> 1
                    lhsT = lhsT_of(p2, a)
                    rhs = wv[64 * a:64 * (a + 1), :]
                    j = (t - t0) >> 1
                    ps = ps_e if a == 0 else ps_o
                    nc.tensor.matmul(ps[:, j * 64:(j + 1) * 64], lhsT, rhs,
                                     start=True, stop=True)

                # bias-add PSUM -> out sbuf (even chunks then odd chunks).
                # Out cols for chunk t0+2j+a are (t0+2j+a)*64 — view the
                # group's columns at pair (128-col) granularity, then slice
                # the even/odd 64-col half of each pair.
                opi, ocol = out_piece_of(t0 * 64)
                ot = out_tiles[opi]
                width = nhalf * 64
                ot_pairs = ot[:, ocol:ocol + nhalf * 128].rearrange(
                    "p (j w) -> p j w", w=128)
                bsrc = bias.unsqueeze(1).broadcast_to([128, nhalf, 64])
                for a, ps in ((0, ps_e), (1, ps_o)):
                    dst = ot_pairs[:, :, a * 64:(a + 1) * 64]
                    src = ps[:, :width].rearrange("p (j k) -> p j k", k=64)
                    nc.vector.tensor_add(dst, src, bsrc)

            # one store per psum-group, issued on the scalar ring as soon
            # as that group's bias-adds land (loads own the sync ring)
            for g in range(n_groups):
                lo = GROUP_BOUNDS[g] * C
                hi = GROUP_BOUNDS[g + 1] * C
                w = hi - lo
                opi, ocol = out_piece_of(lo)
                dst = out_d.ap()[128 * lo:128 * hi].rearrange(
                    "(p w) -> p w", p=128)
                nc.scalar.dma_start(dst, out_tiles[opi][:, ocol:ocol + w])

    nc.compile()
    _cache["nc"] = nc
    return nc


def pack_inputs(x, Wv, bv):
    """Build the per-core [128, 3328] xtw arrays (header + packed x)."""
    x_pad = np.zeros((N_PAD, C), np.float32)
    x_pad[:N] = x
    # xt[core, 64*a + k, p2*128 + p] = x_pad[core*6400 + p*50 + (2*p2+a), k]
    xc = x_pad.reshape(NCORES, 128, PAIRS, 2, C)
    xt = np.ascontiguousarray(xc.transpose(0, 3, 4, 2, 1)).reshape(
        NCORES, 128, XT_COLS)
    xtw = np.empty((NCORES, 128, HDR + XT_COLS), np.float32)
    xtw[:, :64, 0:C] = Wv
    xtw[:, 64:, 0:C] = Wv
    xtw[:, :, C:2 * C] = bv
    xtw[:, :, HDR:] = xt
    return xtw


def kernel(**inputs):
    global LAST_RESULT
    x = np.asarray(inputs["x"], dtype=np.float32)
    Wv = np.asarray(inputs["Wv"], dtype=np.float32)
    bv = np.asarray(inputs["bv"], dtype=np.float32)

    nc = _get_compiled()
    xtw = pack_inputs(x, Wv, bv)

    from concourse.bass_utils import run_bass_kernel_spmd
    in_maps = [{"xtw": xtw[i]} for i in range(NCORES)]
    res = run_bass_kernel_spmd(nc, in_maps, list(range(NCORES)),
                               trace=TRACE)
    LAST_RESULT = res
    out = np.concatenate(
        [unpack_core(res.results[i]["out"]) for i in range(NCORES)],
        axis=0)[:N]
    return np.ascontiguousarray(out)


def unpack_core(flat):
    """Flat group-contiguous device output -> (6400, 64) rows."""
    # group blocks are stored [128, w] p-major back to back
    blocks = []
    for g in range(len(GROUP_SIZES)):
        lo = GROUP_BOUNDS[g] * C
        hi = GROUP_BOUNDS[g + 1] * C
        blocks.append(flat[128 * lo:128 * hi].reshape(128, hi - lo))
    out_sb = np.concatenate(blocks, axis=1)       # [128, (t k)]
    return out_sb.reshape(ROWS_PER_CORE, C)       # row = p*T + t


# revision 53
# speedup vs baseline: 1.0021x; 1.0021x over previous
"""Trainium2 Bass kernel for nn_PointTransformerLayer_59674275611307.

Mathematical simplification: in the reference, the attention logits `w` are
broadcast identically across the NSAMPLE axis before the softmax.  Softmax
over identical values is exactly uniform (1/16 each), and the weights sum to
exactly 1, so the grouped weighted sum of values collapses to the values
themselves:

    out = (xv_g * attn).sum(axis=1) == xv == x @ Wv + bv

(verified: rel err ~2e-7 vs the full reference).  Everything else — the q/k
projections, the position MLP, both BN+MLP stacks and the softmax — cancels
out of the output entirely.  The kernel therefore computes the single
(50000,64)@(64,64) matmul + bias, data-parallel over points across 8 cores.

Device strategy (per core, 6400 rows after padding 50000 -> 51200):
  - host packs the core's shard into ONE dram tensor "xtw" [128, 3328]:
    cols 0:64   = Wv stacked twice on the partition dim (for the two PE
                  row-groups), cols 64:128 = bias replicated, cols 128:3328
    = x transposed in chunk-pairs: partition 64*(t&1)+k, column
    128 + (t>>1)*128 + p holds x[p*50 + t, k]  (t = 128-row chunk index,
    p = row-within-chunk).  The contraction dim k becomes the SBUF
    partition dim (the PE contracts over partitions) with zero on-device
    transposes, full-128-partition DMAs, and contiguous descriptors.
  - 50 fp32 matmuls (lhsT = x-chunk.T [64,128] stationary, rhs = Wv [64,64]
    moving), row-packed in even/odd pairs at tile_position (0,0)/(64,0) so
    consecutive chunks run concurrently in disjoint PE row-groups writing
    separate PSUM banks.
  - dummy bf16 matmuls warm the PE HAM clock gate during the input DMAs.
  - DVE adds the bias while copying PSUM -> SBUF; contiguous stores.
"""

import numpy as np

N = 50000
C = 64
NCORES = 8
T = 50                        # 128-row chunks per core
ROWS_PER_CORE = 128 * T       # 6400
N_PAD = NCORES * ROWS_PER_CORE  # 51200
PAIRS = T // 2                # 25
XT_COLS = PAIRS * 128         # 3200
HDR = 128                     # wv (64) + bias (64) header columns
GROUP = 8                     # max chunks per psum group (4 even + 4 odd)


def _group_sizes():
    sizes = [2, 2, 4]
    while sum(sizes) < T:
        sizes.append(min(GROUP, T - sum(sizes)))
    return sizes


GROUP_SIZES = _group_sizes()
GROUP_BOUNDS = [0]
for _s in GROUP_SIZES:
    GROUP_BOUNDS.append(GROUP_BOUNDS[-1] + _s)
# xt load pieces in pairs-of-chunks; piece 0 additionally carries the header
XT_PIECE_PAIRS = [4, 7, 7, 7]
# Output store pieces (columns of the [128, 3200] out-sbuf layout).
OUT_PIECES = [(0, 1024), (1024, 2048), (2048, 3072), (3072, 3200)]
N_WARMUP = 12  # dummy bf16 matmuls to engage the PE HAM clock during DMA-in

TRACE = False          # test.py sets True to collect an NTFF profile
LAST_RESULT = None     # BassKernelResults of the last run (for test.py)

_cache = {}


def _get_compiled():
    if "nc" in _cache:
        return _cache["nc"]

    import concourse.mybir as mybir
    import concourse.tile as tile
    from concourse import bacc
    from concourse.bass import MemorySpace

    f32 = mybir.dt.float32
    bf16 = mybir.dt.bfloat16
    nc = bacc.Bacc("TRN2", target_bir_lowering=False, debug=False,
                   num_devices=NCORES)

    xtw_d = nc.dram_tensor("xtw", [128, HDR + XT_COLS], f32,
                           kind="ExternalInput")
    # Output stored group-contiguous: store s occupies a fully contiguous
    # DRAM region [128, w_s] (HBM writes at 12.8KB partition stride measured
    # only ~190GB/s; contiguous pieces avoid that).  Host reassembles.
    out_d = nc.dram_tensor("out", [128 * XT_COLS], f32,
                           kind="ExternalOutput")

    with tile.TileContext(nc) as tc:
        with (
            tc.tile_pool(name="const", bufs=1) as constp,
            tc.tile_pool(name="xt", bufs=1) as xtp,
            tc.tile_pool(name="outp", bufs=1) as outp,
            tc.tile_pool(name="ps", bufs=3, space=MemorySpace.PSUM) as psp,
        ):
            # PE warmup: dummy bf16 matmuls gated only on a gpsimd memset,
            # so they run during the input-DMA window and the HAM clock
            # gate reaches 8/8 before the real (fp32) matmul stream starts.
            scr = constp.tile([128, 384], bf16, tag="scr")
            nc.gpsimd.memset(scr[:], 0.0)
            ps_w = psp.tile([128, 512], f32, tag="warm", bufs=1)
            for _ in range(N_WARMUP):
                nc.tensor.matmul(ps_w[:, :384], scr[:, :128], scr[:],
                                 start=True, stop=True)

            # Input DMAs: all pieces on the sync HWDGE ring, in consumption
            # order — the ring drains FIFO, so piece 0 (header + first
            # pairs) completes first and the matmul stream starts early.
            # (Splitting across rings makes every piece finish near the end
            # because the SDMA engines round-robin between rings.)
            xt_tiles = []   # (tile, first_pair, col_off_of_first_pair)
            col = 0
            for i, npair in enumerate(XT_PIECE_PAIRS):
                w = npair * 128 + (HDR if i == 0 else 0)
                t_ = xtp.tile([128, w], f32, tag=f"xt{i}", name=f"xt_sb{i}")
                nc.sync.dma_start(t_[:], xtw_d.ap()[:, col:col + w])
                first_pair = 0 if i == 0 else (col - HDR) // 128
                xt_tiles.append((t_, first_pair, HDR if i == 0 else 0))
                col += w
            wv = xt_tiles[0][0][:, 0:C]
            bias = xt_tiles[0][0][:, C:2 * C]

            def lhsT_of(p2, a):
                for t_, first, off in xt_tiles:
                    npair = (t_.shape[1] - off) // 128
                    if first <= p2 < first + npair:
                        local = off + (p2 - first) * 128
                        return t_[64 * a:64 * (a + 1), local:local + 128]
                raise AssertionError(p2)

            out_tiles = []
            for i, (lo, hi) in enumerate(OUT_PIECES):
                out_tiles.append(
                    outp.tile([128, hi - lo], f32, tag=f"out{i}",
                              name=f"out_sb{i}"))

            def out_piece_of(col):
                for i, (lo, hi) in enumerate(OUT_PIECES):
                    if lo <= col < hi:
                        return i, col - lo
                raise AssertionError(col)

            # group sizes ramp up: small first groups let the first store
            # (and the slow ~225GB/s HBM write stream) start ~1.5us earlier
            n_groups = len(GROUP_SIZES)
            for g in range(n_groups):
                t0 = GROUP_BOUNDS[g]
                t1 = GROUP_BOUNDS[g + 1]
                nhalf = (t1 - t0) // 2          # chunks per parity
                ps_e = psp.tile([128, 256], f32, tag="mme")
                ps_o = psp.tile([128, 256], f32, tag="mmo")
                for t in range(t0, t1):
                    a = t & 1
                    p2 = t >> 1
                    lhsT = lhsT_of(p2, a)
                    rhs = wv[64 * a:64 * (a + 1), :]
                    j = (t - t0) >> 1
                    ps = ps_e if a == 0 else ps_o
                    nc.tensor.matmul(ps[:, j * 64:(j + 1) * 64], lhsT, rhs,
                                     start=True, stop=True)

                # bias-add PSUM -> out sbuf (even chunks then odd chunks).
                # Out cols for chunk t0+2j+a are (t0+2j+a)*64 — view the
                # group's columns at pair (128-col) granularity, then slice
                # the even/odd 64-col half of each pair.
                opi, ocol = out_piece_of(t0 * 64)
                ot = out_tiles[opi]
                width = nhalf * 64
                ot_pairs = ot[:, ocol:ocol + nhalf * 128].rearrange(
                    "p (j w) -> p j w", w=128)
                bsrc = bias.unsqueeze(1).broadcast_to([128, nhalf, 64])
                for a, ps in ((0, ps_e), (1, ps_o)):
                    dst = ot_pairs[:, :, a * 64:(a + 1) * 64]
                    src = ps[:, :width].rearrange("p (j k) -> p j k", k=64)
                    nc.vector.tensor_add(dst, src, bsrc)

            # one store per psum-group, issued on the scalar ring as soon
            # as that group's bias-adds land (loads own the sync ring)
            for g in range(n_groups):
                lo = GROUP_BOUNDS[g] * C
                hi = GROUP_BOUNDS[g + 1] * C
                w = hi - lo
                opi, ocol = out_piece_of(lo)
                dst = out_d.ap()[128 * lo:128 * hi].rearrange(
                    "(p w) -> p w", p=128)
                nc.scalar.dma_start(dst, out_tiles[opi][:, ocol:ocol + w])

    nc.compile()
    _cache["nc"] = nc
    return nc


def pack_inputs(x, Wv, bv):
    """Build the per-core [128, 3328] xtw arrays (header + packed x)."""
    x_pad = np.zeros((N_PAD, C), np.float32)
    x_pad[:N] = x
    # xt[core, 64*a + k, p2*128 + p] = x_pad[core*6400 + p*50 + (2*p2+a), k]
    xc = x_pad.reshape(NCORES, 128, PAIRS, 2, C)
    xt = np.ascontiguousarray(xc.transpose(0, 3, 4, 2, 1)).reshape(
        NCORES, 128, XT_COLS)
    xtw = np.empty((NCORES, 128, HDR + XT_COLS), np.float32)
    xtw[:, :64, 0:C] = Wv
    xtw[:, 64:, 0:C] = Wv
    xtw[:, :, C:2 * C] = bv
    xtw[:, :, HDR:] = xt
    return xtw


def kernel(**inputs):
    global LAST_RESULT
    x = np.asarray(inputs["x"], dtype=np.float32)
    Wv = np.asarray(inputs["Wv"], dtype=np.float32)
    bv = np.asarray(inputs["bv"], dtype=np.float32)

    nc = _get_compiled()
    xtw = pack_inputs(x, Wv, bv)

    from concourse.bass_utils import run_bass_kernel_spmd
    in_maps = [{"xtw": xtw[i]} for i in range(NCORES)]
    res = run_bass_kernel_spmd(nc, in_maps, list(range(NCORES)),
                               trace=TRACE)
    LAST_RESULT = res
    out = np.concatenate(
        [unpack_core(res.results[i]["out"]) for i in range(NCORES)],
        axis=0)[:N]
    return np.ascontiguousarray(out)


def unpack_core(flat):
    """Flat group-contiguous device output -> (6400, 64) rows."""
    # group blocks are stored [128, w] p-major back to back
    blocks = []
    for g in range(len(GROUP_SIZES)):
        lo = GROUP_BOUNDS[g] * C
        hi = GROUP_BOUNDS[g + 1] * C
        blocks.append(flat[128 * lo:128 * hi].reshape(128, hi - lo))
    out_sb = np.concatenate(blocks, axis=1)       # [128, (t k)]
    return out_sb.reshape(ROWS_PER_CORE, C)       # row = p*T + t


# revision 54
# speedup vs baseline: 1.0373x; 1.0352x over previous
"""Trainium2 Bass kernel for nn_PointTransformerLayer_59674275611307.

Mathematical simplification: in the reference, the attention logits `w` are
broadcast identically across the NSAMPLE axis before the softmax.  Softmax
over identical values is exactly uniform (1/16 each), and the weights sum to
exactly 1, so the grouped weighted sum of values collapses to the values
themselves:

    out = (xv_g * attn).sum(axis=1) == xv == x @ Wv + bv

(verified: rel err ~2e-7 vs the full reference).  Everything else — the q/k
projections, the position MLP, both BN+MLP stacks and the softmax — cancels
out of the output entirely.  The kernel therefore computes the single
(50000,64)@(64,64) matmul + bias, data-parallel over points across 8 cores.

Device strategy (per core, 6400 rows after padding 50000 -> 51200):
  - host packs the core's shard into ONE dram tensor "xtw" [128, 3328]:
    cols 0:64   = Wv stacked twice on the partition dim (for the two PE
                  row-groups), cols 64:128 = bias replicated, cols 128:3328
    = x transposed in chunk-pairs: partition 64*(t&1)+k, column
    128 + (t>>1)*128 + p holds x[p*50 + t, k]  (t = 128-row chunk index,
    p = row-within-chunk).  The contraction dim k becomes the SBUF
    partition dim (the PE contracts over partitions) with zero on-device
    transposes, full-128-partition DMAs, and contiguous descriptors.
  - 50 fp32 matmuls (lhsT = x-chunk.T [64,128] stationary, rhs = Wv [64,64]
    moving), row-packed in even/odd pairs at tile_position (0,0)/(64,0) so
    consecutive chunks run concurrently in disjoint PE row-groups writing
    separate PSUM banks.
  - dummy bf16 matmuls warm the PE HAM clock gate during the input DMAs.
  - DVE adds the bias while copying PSUM -> SBUF; contiguous stores.
"""

import numpy as np

N = 50000
C = 64
NCORES = 8
T = 50                        # 128-row chunks per core
ROWS_PER_CORE = 128 * T       # 6400
N_PAD = NCORES * ROWS_PER_CORE  # 51200
PAIRS = T // 2                # 25
XT_COLS = PAIRS * 128         # 3200
HDR = 128                     # wv (64) + bias (64) header columns
GROUP = 8                     # max chunks per psum group (4 even + 4 odd)


def _group_sizes():
    sizes = [2, 2, 4]
    while sum(sizes) < T:
        sizes.append(min(GROUP, T - sum(sizes)))
    return sizes


GROUP_SIZES = _group_sizes()
GROUP_BOUNDS = [0]
for _s in GROUP_SIZES:
    GROUP_BOUNDS.append(GROUP_BOUNDS[-1] + _s)
# xt load pieces in pairs-of-chunks; piece 0 additionally carries the header
XT_PIECE_PAIRS = [4, 7, 7, 7]
# Output store pieces (columns of the [128, 3200] out-sbuf layout).
OUT_PIECES = [(0, 1024), (1024, 2048), (2048, 3072), (3072, 3200)]
N_WARMUP = 12  # dummy bf16 matmuls to engage the PE HAM clock during DMA-in

TRACE = False          # test.py sets True to collect an NTFF profile
LAST_RESULT = None     # BassKernelResults of the last run (for test.py)

_cache = {}


def _get_compiled():
    if "nc" in _cache:
        return _cache["nc"]

    import concourse.mybir as mybir
    import concourse.tile as tile
    from concourse import bacc
    from concourse.bass import MemorySpace

    f32 = mybir.dt.float32
    bf16 = mybir.dt.bfloat16
    nc = bacc.Bacc("TRN2", target_bir_lowering=False, debug=False,
                   num_devices=NCORES)

    xtw_d = nc.dram_tensor("xtw", [128, HDR + XT_COLS], f32,
                           kind="ExternalInput")
    # Output stored group-contiguous: store s occupies a fully contiguous
    # DRAM region [128, w_s] (HBM writes at 12.8KB partition stride measured
    # only ~190GB/s; contiguous pieces avoid that).  Host reassembles.
    out_d = nc.dram_tensor("out", [128 * XT_COLS], f32,
                           kind="ExternalOutput")

    with tile.TileContext(nc) as tc:
        with (
            tc.tile_pool(name="const", bufs=1) as constp,
            tc.tile_pool(name="xt", bufs=1) as xtp,
            tc.tile_pool(name="outp", bufs=1) as outp,
            tc.tile_pool(name="ps", bufs=3, space=MemorySpace.PSUM) as psp,
        ):
            # PE warmup: dummy bf16 matmuls gated only on a gpsimd memset,
            # so they run during the input-DMA window and the HAM clock
            # gate reaches 8/8 before the real (fp32) matmul stream starts.
            scr = constp.tile([128, 384], bf16, tag="scr")
            nc.gpsimd.memset(scr[:], 0.0)
            ps_w = psp.tile([128, 512], f32, tag="warm", bufs=1)
            for _ in range(N_WARMUP):
                nc.tensor.matmul(ps_w[:, :384], scr[:, :128], scr[:],
                                 start=True, stop=True)

            # Input DMAs: all pieces on the sync HWDGE ring, in consumption
            # order — the ring drains FIFO, so piece 0 (header + first
            # pairs) completes first and the matmul stream starts early.
            # (Splitting across rings makes every piece finish near the end
            # because the SDMA engines round-robin between rings.)
            xt_tiles = []   # (tile, first_pair, col_off_of_first_pair)
            col = 0
            for i, npair in enumerate(XT_PIECE_PAIRS):
                w = npair * 128 + (HDR if i == 0 else 0)
                t_ = xtp.tile([128, w], f32, tag=f"xt{i}", name=f"xt_sb{i}")
                nc.sync.dma_start(t_[:], xtw_d.ap()[:, col:col + w])
                first_pair = 0 if i == 0 else (col - HDR) // 128
                xt_tiles.append((t_, first_pair, HDR if i == 0 else 0))
                col += w
            wv = xt_tiles[0][0][:, 0:C]
            bias = xt_tiles[0][0][:, C:2 * C]

            def lhsT_of(p2, a):
                for t_, first, off in xt_tiles:
                    npair = (t_.shape[1] - off) // 128
                    if first <= p2 < first + npair:
                        local = off + (p2 - first) * 128
                        return t_[64 * a:64 * (a + 1), local:local + 128]
                raise AssertionError(p2)

            out_tiles = []
            for i, (lo, hi) in enumerate(OUT_PIECES):
                out_tiles.append(
                    outp.tile([128, hi - lo], f32, tag=f"out{i}",
                              name=f"out_sb{i}"))

            def out_piece_of(col):
                for i, (lo, hi) in enumerate(OUT_PIECES):
                    if lo <= col < hi:
                        return i, col - lo
                raise AssertionError(col)

            # group sizes ramp up: small first groups let the first store
            # (and the slow ~225GB/s HBM write stream) start ~1.5us earlier
            n_groups = len(GROUP_SIZES)
            for g in range(n_groups):
                t0 = GROUP_BOUNDS[g]
                t1 = GROUP_BOUNDS[g + 1]
                nhalf = (t1 - t0) // 2          # chunks per parity
                ps_e = psp.tile([128, 256], f32, tag="mme")
                ps_o = psp.tile([128, 256], f32, tag="mmo")
                for t in range(t0, t1):
                    a = t & 1
                    p2 = t >> 1
                    lhsT = lhsT_of(p2, a)
                    rhs = wv[64 * a:64 * (a + 1), :]
                    j = (t - t0) >> 1
                    ps = ps_e if a == 0 else ps_o
                    nc.tensor.matmul(ps[:, j * 64:(j + 1) * 64], lhsT, rhs,
                                     start=True, stop=True)

                # bias-add PSUM -> out sbuf (even chunks then odd chunks).
                # Out cols for chunk t0+2j+a are (t0+2j+a)*64 — view the
                # group's columns at pair (128-col) granularity, then slice
                # the even/odd 64-col half of each pair.
                opi, ocol = out_piece_of(t0 * 64)
                ot = out_tiles[opi]
                width = nhalf * 64
                ot_pairs = ot[:, ocol:ocol + nhalf * 128].rearrange(
                    "p (j w) -> p j w", w=128)
                bsrc = bias.unsqueeze(1).broadcast_to([128, nhalf, 64])
                for a, ps in ((0, ps_e), (1, ps_o)):
                    dst = ot_pairs[:, :, a * 64:(a + 1) * 64]
                    src = ps[:, :width].rearrange("p (j k) -> p j k", k=64)
                    nc.vector.tensor_add(dst, src, bsrc)

            # one store per psum-group, issued on the scalar ring as soon
            # as that group's bias-adds land (loads own the sync ring)
            for g in range(n_groups):
                lo = GROUP_BOUNDS[g] * C
                hi = GROUP_BOUNDS[g + 1] * C
                w = hi - lo
                opi, ocol = out_piece_of(lo)
                dst = out_d.ap()[128 * lo:128 * hi].rearrange(
                    "(p w) -> p w", p=128)
                # alternate the scalar HWDGE ring and the (otherwise idle)
                # gpsimd SWDGE path — two independent descriptor streams
                # push the ~225GB/s store wall higher; only the last
                # store's completion matters
                eng = nc.scalar if g % 2 == 0 else nc.gpsimd
                eng.dma_start(dst, out_tiles[opi][:, ocol:ocol + w])

    nc.compile()
    _cache["nc"] = nc
    return nc


def pack_inputs(x, Wv, bv):
    """Build the per-core [128, 3328] xtw arrays (header + packed x)."""
    x_pad = np.zeros((N_PAD, C), np.float32)
    x_pad[:N] = x
    # xt[core, 64*a + k, p2*128 + p] = x_pad[core*6400 + p*50 + (2*p2+a), k]
    xc = x_pad.reshape(NCORES, 128, PAIRS, 2, C)
    xt = np.ascontiguousarray(xc.transpose(0, 3, 4, 2, 1)).reshape(
        NCORES, 128, XT_COLS)
    xtw = np.empty((NCORES, 128, HDR + XT_COLS), np.float32)
    xtw[:, :64, 0:C] = Wv
    xtw[:, 64:, 0:C] = Wv
    xtw[:, :, C:2 * C] = bv
    xtw[:, :, HDR:] = xt
    return xtw


def kernel(**inputs):
    global LAST_RESULT
    x = np.asarray(inputs["x"], dtype=np.float32)
    Wv = np.asarray(inputs["Wv"], dtype=np.float32)
    bv = np.asarray(inputs["bv"], dtype=np.float32)

    nc = _get_compiled()
    xtw = pack_inputs(x, Wv, bv)

    from concourse.bass_utils import run_bass_kernel_spmd
    in_maps = [{"xtw": xtw[i]} for i in range(NCORES)]
    res = run_bass_kernel_spmd(nc, in_maps, list(range(NCORES)),
                               trace=TRACE)
    LAST_RESULT = res
    out = np.concatenate(
        [unpack_core(res.results[i]["out"]) for i in range(NCORES)],
        axis=0)[:N]
    return np.ascontiguousarray(out)


def unpack_core(flat):
    """Flat group-contiguous device output -> (6400, 64) rows."""
    # group blocks are stored [128, w] p-major back to back
    blocks = []
    for g in range(len(GROUP_SIZES)):
        lo = GROUP_BOUNDS[g] * C
        hi = GROUP_BOUNDS[g + 1] * C
        blocks.append(flat[128 * lo:128 * hi].reshape(128, hi - lo))
    out_sb = np.concatenate(blocks, axis=1)       # [128, (t k)]
    return out_sb.reshape(ROWS_PER_CORE, C)       # row = p*T + t


# revision 55
# speedup vs baseline: 1.1785x; 1.1361x over previous
"""Trainium2 Bass kernel for nn_PointTransformerLayer_59674275611307.

Mathematical simplification: in the reference, the attention logits `w` are
broadcast identically across the NSAMPLE axis before the softmax.  Softmax
over identical values is exactly uniform (1/16 each), and the weights sum to
exactly 1, so the grouped weighted sum of values collapses to the values
themselves:

    out = (xv_g * attn).sum(axis=1) == xv == x @ Wv + bv

(verified: rel err ~2e-7 vs the full reference).  Everything else — the q/k
projections, the position MLP, both BN+MLP stacks and the softmax — cancels
out of the output entirely.  The kernel therefore computes the single
(50000,64)@(64,64) matmul + bias, data-parallel over points across 8 cores.

Device strategy (per core, 6400 rows after padding 50000 -> 51200):
  - host packs the core's shard into ONE dram tensor "xtw" [128, 3328]:
    cols 0:64   = Wv stacked twice on the partition dim (for the two PE
                  row-groups), cols 64:128 = bias replicated, cols 128:3328
    = x transposed in chunk-pairs: partition 64*(t&1)+k, column
    128 + (t>>1)*128 + p holds x[p*50 + t, k]  (t = 128-row chunk index,
    p = row-within-chunk).  The contraction dim k becomes the SBUF
    partition dim (the PE contracts over partitions) with zero on-device
    transposes, full-128-partition DMAs, and contiguous descriptors.
  - 50 fp32 matmuls (lhsT = x-chunk.T [64,128] stationary, rhs = Wv [64,64]
    moving), row-packed in even/odd pairs at tile_position (0,0)/(64,0) so
    consecutive chunks run concurrently in disjoint PE row-groups writing
    separate PSUM banks.
  - dummy bf16 matmuls warm the PE HAM clock gate during the input DMAs.
  - DVE adds the bias while copying PSUM -> SBUF; contiguous stores.
"""

import numpy as np

N = 50000
C = 64
NCORES = 8
T = 50                        # 128-row chunks per core
ROWS_PER_CORE = 128 * T       # 6400
N_PAD = NCORES * ROWS_PER_CORE  # 51200
PAIRS = T // 2                # 25
XT_COLS = PAIRS * 128         # 3200
HDR = 128                     # wv (64) + bias (64) header columns
GROUP = 8                     # max chunks per psum group (4 even + 4 odd)


def _group_sizes():
    sizes = [2, 2, 4]
    while sum(sizes) < T:
        sizes.append(min(GROUP, T - sum(sizes)))
    return sizes


GROUP_SIZES = _group_sizes()
GROUP_BOUNDS = [0]
for _s in GROUP_SIZES:
    GROUP_BOUNDS.append(GROUP_BOUNDS[-1] + _s)
# xt load pieces in pairs-of-chunks; piece 0 additionally carries the header
XT_PIECE_PAIRS = [4, 7, 7, 7]
# Output store pieces (columns of the [128, 3200] out-sbuf layout).
OUT_PIECES = [(0, 1024), (1024, 2048), (2048, 3072), (3072, 3200)]
N_WARMUP = 12  # dummy bf16 matmuls to engage the PE HAM clock during DMA-in

TRACE = False          # test.py sets True to collect an NTFF profile
LAST_RESULT = None     # BassKernelResults of the last run (for test.py)

_cache = {}


def _get_compiled():
    if "nc" in _cache:
        return _cache["nc"]

    import concourse.mybir as mybir
    import concourse.tile as tile
    from concourse import bacc
    from concourse.bass import MemorySpace

    f32 = mybir.dt.float32
    bf16 = mybir.dt.bfloat16
    nc = bacc.Bacc("TRN2", target_bir_lowering=False, debug=False,
                   num_devices=NCORES)

    xtw_d = nc.dram_tensor("xtw", [128, HDR + XT_COLS], f32,
                           kind="ExternalInput")
    # Output stored group-contiguous: store s occupies a fully contiguous
    # DRAM region [128, w_s] (HBM writes at 12.8KB partition stride measured
    # only ~190GB/s; contiguous pieces avoid that).  Host reassembles.
    out_d = nc.dram_tensor("out", [128 * XT_COLS], f32,
                           kind="ExternalOutput")

    with tile.TileContext(nc) as tc:
        with (
            tc.tile_pool(name="const", bufs=1) as constp,
            tc.tile_pool(name="xt", bufs=1) as xtp,
            tc.tile_pool(name="outp", bufs=1) as outp,
            tc.tile_pool(name="ps", bufs=3, space=MemorySpace.PSUM) as psp,
        ):
            # PE warmup: dummy bf16 matmuls gated only on a gpsimd memset,
            # so they run during the input-DMA window and the HAM clock
            # gate reaches 8/8 before the real (fp32) matmul stream starts.
            scr = constp.tile([128, 384], bf16, tag="scr")
            nc.gpsimd.memset(scr[:], 0.0)
            ps_w = psp.tile([128, 512], f32, tag="warm", bufs=1)
            for _ in range(N_WARMUP):
                nc.tensor.matmul(ps_w[:, :384], scr[:, :128], scr[:],
                                 start=True, stop=True)

            # Input DMAs: all pieces on the sync HWDGE ring, in consumption
            # order — the ring drains FIFO, so piece 0 (header + first
            # pairs) completes first and the matmul stream starts early.
            # (Splitting across rings makes every piece finish near the end
            # because the SDMA engines round-robin between rings.)
            xt_tiles = []   # (tile, first_pair, col_off_of_first_pair)
            col = 0
            for i, npair in enumerate(XT_PIECE_PAIRS):
                w = npair * 128 + (HDR if i == 0 else 0)
                t_ = xtp.tile([128, w], f32, tag=f"xt{i}", name=f"xt_sb{i}")
                nc.sync.dma_start(t_[:], xtw_d.ap()[:, col:col + w])
                first_pair = 0 if i == 0 else (col - HDR) // 128
                xt_tiles.append((t_, first_pair, HDR if i == 0 else 0))
                col += w
            wv = xt_tiles[0][0][:, 0:C]
            bias = xt_tiles[0][0][:, C:2 * C]

            def lhsT_of(p2, a):
                for t_, first, off in xt_tiles:
                    npair = (t_.shape[1] - off) // 128
                    if first <= p2 < first + npair:
                        local = off + (p2 - first) * 128
                        return t_[64 * a:64 * (a + 1), local:local + 128]
                raise AssertionError(p2)

            out_tiles = []
            for i, (lo, hi) in enumerate(OUT_PIECES):
                out_tiles.append(
                    outp.tile([128, hi - lo], f32, tag=f"out{i}",
                              name=f"out_sb{i}"))

            def out_piece_of(col):
                for i, (lo, hi) in enumerate(OUT_PIECES):
                    if lo <= col < hi:
                        return i, col - lo
                raise AssertionError(col)

            # group sizes ramp up: small first groups let the first store
            # (and the slow ~225GB/s HBM write stream) start ~1.5us earlier
            n_groups = len(GROUP_SIZES)
            for g in range(n_groups):
                t0 = GROUP_BOUNDS[g]
                t1 = GROUP_BOUNDS[g + 1]
                nhalf = (t1 - t0) // 2          # chunks per parity
                ps_e = psp.tile([128, 256], f32, tag="mme")
                ps_o = psp.tile([128, 256], f32, tag="mmo")
                for t in range(t0, t1):
                    a = t & 1
                    p2 = t >> 1
                    lhsT = lhsT_of(p2, a)
                    rhs = wv[64 * a:64 * (a + 1), :]
                    j = (t - t0) >> 1
                    ps = ps_e if a == 0 else ps_o
                    nc.tensor.matmul(ps[:, j * 64:(j + 1) * 64], lhsT, rhs,
                                     start=True, stop=True)

                # bias-add PSUM -> out sbuf (even chunks then odd chunks).
                # Out cols for chunk t0+2j+a are (t0+2j+a)*64 — view the
                # group's columns at pair (128-col) granularity, then slice
                # the even/odd 64-col half of each pair.
                opi, ocol = out_piece_of(t0 * 64)
                ot = out_tiles[opi]
                width = nhalf * 64
                ot_pairs = ot[:, ocol:ocol + nhalf * 128].rearrange(
                    "p (j w) -> p j w", w=128)
                bsrc = bias.unsqueeze(1).broadcast_to([128, nhalf, 64])
                for a, ps in ((0, ps_e), (1, ps_o)):
                    dst = ot_pairs[:, :, a * 64:(a + 1) * 64]
                    src = ps[:, :width].rearrange("p (j k) -> p j k", k=64)
                    nc.vector.tensor_add(dst, src, bsrc)

            # one store per psum-group, issued on the scalar ring as soon
            # as that group's bias-adds land (loads own the sync ring)
            for g in range(n_groups):
                lo = GROUP_BOUNDS[g] * C
                hi = GROUP_BOUNDS[g + 1] * C
                w = hi - lo
                opi, ocol = out_piece_of(lo)
                dst = out_d.ap()[128 * lo:128 * hi].rearrange(
                    "(p w) -> p w", p=128)
                nc.scalar.dma_start(dst, out_tiles[opi][:, ocol:ocol + w])

    nc.compile()
    _cache["nc"] = nc
    return nc


def pack_inputs(x, Wv, bv):
    """Build the per-core [128, 3328] xtw arrays (header + packed x)."""
    x_pad = np.zeros((N_PAD, C), np.float32)
    x_pad[:N] = x
    # xt[core, 64*a + k, p2*128 + p] = x_pad[core*6400 + p*50 + (2*p2+a), k]
    xc = x_pad.reshape(NCORES, 128, PAIRS, 2, C)
    xt = np.ascontiguousarray(xc.transpose(0, 3, 4, 2, 1)).reshape(
        NCORES, 128, XT_COLS)
    xtw = np.empty((NCORES, 128, HDR + XT_COLS), np.float32)
    xtw[:, :64, 0:C] = Wv
    xtw[:, 64:, 0:C] = Wv
    xtw[:, :, C:2 * C] = bv
    xtw[:, :, HDR:] = xt
    return xtw


def kernel(**inputs):
    global LAST_RESULT
    x = np.asarray(inputs["x"], dtype=np.float32)
    Wv = np.asarray(inputs["Wv"], dtype=np.float32)
    bv = np.asarray(inputs["bv"], dtype=np.float32)

    nc = _get_compiled()
    xtw = pack_inputs(x, Wv, bv)

    from concourse.bass_utils import run_bass_kernel_spmd
    in_maps = [{"xtw": xtw[i]} for i in range(NCORES)]
    res = run_bass_kernel_spmd(nc, in_maps, list(range(NCORES)),
                               trace=TRACE)
    LAST_RESULT = res
    out = np.concatenate(
        [unpack_core(res.results[i]["out"]) for i in range(NCORES)],
        axis=0)[:N]
    return np.ascontiguousarray(out)


def unpack_core(flat):
    """Flat group-contiguous device output -> (6400, 64) rows."""
    # group blocks are stored [128, w] p-major back to back
    blocks = []
    for g in range(len(GROUP_SIZES)):
        lo = GROUP_BOUNDS[g] * C
        hi = GROUP_BOUNDS[g + 1] * C
        blocks.append(flat[128 * lo:128 * hi].reshape(128, hi - lo))
    out_sb = np.concatenate(blocks, axis=1)       # [128, (t k)]
    return out_sb.reshape(ROWS_PER_CORE, C)       # row = p*T + t
